# revision 14
# baseline (speedup 1.0000x reference)
"""Trainium2 Bass kernel for the noisy top-2-experts MoE router.

Contract: kernel(**inputs) takes the FULL inputs
    inputs: [G=16, S=1024, D=4096] f32
    W:      [D=4096, E=64] f32
    noise:  [G=16, S=1024, E=64] f32
and returns the full reference outputs
    (combine [G,S,E,C], aux_loss [G], importance_loss [G], load_loss [G],
     gates_noisy [G,S,E])

Sharding: group axis G across 8 NeuronCores (2 groups per core); the small
gating weight W is replicated.  Inputs are pre-transposed per group on the
host to [D, S] so the gating GEMM contracts over the partition axis without
any on-device transpose.

Device algorithm per group:
  1. logits = x @ W via PE, accumulating [128 tok, 64 exp] slices packed as
     one [128, 512] PSUM bank per group.
  2. Per 128-token chunk: noisy logits, Max8/MaxIndex top-2, exp/softmax on
     ACT (accum_out gives the softmax denominator), importance & load-loss
     partial sums accumulated with tiny PE matmuls, and expert one-hots for
     the dispatcher.
  3. Dispatcher positions via cumulative-sum-as-matmul: an upper-triangular
     constant U gives the within-chunk inclusive cumsum; a K=1 rank-1 matmul
     adds the running per-expert prefix.
  4. The huge sparse combine tensor is never densified: each selection's
     64-float capacity row is scattered with indirect DMA into the
     pre-zeroed DRAM output (out-of-bounds row index == dropped token).
"""

import math

import numpy as np

F32 = None  # filled lazily in _imports
_IMPORTS = {}


def _imports():
    global F32
    if _IMPORTS:
        return _IMPORTS
    import concourse.bacc as bacc
    import concourse.bass as bass
    import concourse.mybir as mybir
    import concourse.tile as tile
    from concourse.bass_utils import run_bass_kernel_spmd

    _IMPORTS.update(
        bacc=bacc, bass=bass, mybir=mybir, tile=tile,
        run_bass_kernel_spmd=run_bass_kernel_spmd,
    )
    F32 = mybir.dt.float32
    return _IMPORTS


N_CORES = 8
E = 64          # experts
C = 64          # capacity
TOPK = 2
SIGMA_INV = float(E)                       # 1 / (NOISE_STD / NUM_EXPERTS)
ERF_SCALE = float(E) / math.sqrt(2.0)      # z -> erf argument scale
P = 128


def make_consts(S, D):
    """Constant tensors shipped to every core."""
    DC = D // P
    u128 = np.triu(np.ones((P, P), dtype=np.float32))          # U[j,i]=1 if j<=i
    eye64 = np.eye(E, dtype=np.float32)
    iota_row = np.tile(np.arange(C, dtype=np.float32), (P, 1))  # [128, 64]
    iotap = (np.arange(P, dtype=np.float32) * E)[:, None]       # [128, 1] p*64
    ones_col = np.ones((P, 1), dtype=np.float32)
    ones_row = np.ones((1, P), dtype=np.float32)
    post_scale = np.array([[1.0], [0.5 / S]], dtype=np.float32)
    post_bias = np.array([[0.0], [0.5]], dtype=np.float32)
    return {
        "u128": u128, "eye64": eye64, "iota_row": iota_row, "iotap": iotap,
        "ones_col": ones_col, "ones_row": ones_row,
        "post_scale": post_scale, "post_bias": post_bias,
    }


def build_nc(gpc, S, D, zero_fill=False, erf_func=None):
    """Build + compile the per-core Bass program.

    gpc: groups per core; S: tokens per group; D: model dim.
    zero_fill: also zero the combine output from the device (fallback when
    pre-zeroed ExternalOutput buffers can't be relied on).
    """
    m = _imports()
    bacc, bass, mybir, tile = m["bacc"], m["bass"], m["mybir"], m["tile"]
    AF = mybir.ActivationFunctionType
    if erf_func is None:
        erf_func = AF.Erf  # CoreSim lacks Erf; tests may substitute Tanh
    OP = mybir.AluOpType
    AX = mybir.AxisListType
    f32 = mybir.dt.float32
    i32 = mybir.dt.int32
    u32 = mybir.dt.uint32

    SC = S // P     # token chunks per group
    DC = D // P     # contraction chunks
    NROW = gpc * S * E   # combine rows per core

    nc = bacc.Bacc(
        "TRN2", target_bir_lowering=False, debug=False,
        enable_asserts=False,
    )

    xT_d = nc.dram_tensor("xT", [gpc, D, S], f32, kind="ExternalInput")
    noise_d = nc.dram_tensor("noiseg", [gpc, S, E], f32, kind="ExternalInput")
    w_d = nc.dram_tensor("Wg", [D, E], f32, kind="ExternalInput")
    u_d = nc.dram_tensor("u128", [P, P], f32, kind="ExternalInput")
    eye_d = nc.dram_tensor("eye64", [E, E], f32, kind="ExternalInput")
    iotar_d = nc.dram_tensor("iota_row", [P, C], f32, kind="ExternalInput")
    iotap_d = nc.dram_tensor("iotap", [P, 1], f32, kind="ExternalInput")
    onesc_d = nc.dram_tensor("ones_col", [P, 1], f32, kind="ExternalInput")
    onesr_d = nc.dram_tensor("ones_row", [1, P], f32, kind="ExternalInput")
    pscale_d = nc.dram_tensor("post_scale", [2, 1], f32, kind="ExternalInput")
    pbias_d = nc.dram_tensor("post_bias", [2, 1], f32, kind="ExternalInput")

    combine_d = nc.dram_tensor("combine", [NROW, C], f32, kind="ExternalOutput")
    gates_d = nc.dram_tensor("gates_noisy", [gpc, S, E], f32, kind="ExternalOutput")
    imp_d = nc.dram_tensor("imp_out", [gpc, 1], f32, kind="ExternalOutput")
    load_d = nc.dram_tensor("load_out", [gpc, 1], f32, kind="ExternalOutput")
    aux_d = nc.dram_tensor("aux_out", [gpc, 1], f32, kind="ExternalOutput")

    with tile.TileContext(nc) as tc:
        with tc.tile_pool(name="sb", bufs=1) as sb, \
             tc.tile_pool(name="ps", bufs=1, space="PSUM") as ps:

            # ---- persistent constants ----
            u_t = sb.tile([P, P], f32, tag="u128")
            eye_t = sb.tile([E, E], f32, tag="eye64")
            iotar_t = sb.tile([P, C], f32, tag="iotar")
            iotap_t = sb.tile([P, 1], f32, tag="iotap")
            onesc_t = sb.tile([P, 1], f32, tag="onesc")
            onesr_t = sb.tile([1, P], f32, tag="onesr")
            pscale_t = sb.tile([2, 1], f32, tag="pscale")
            pbias_t = sb.tile([2, 1], f32, tag="pbias")
            w_t = sb.tile([P, DC * E], f32, tag="w")
            nc.sync.dma_start(out=u_t[:], in_=u_d[:])
            nc.sync.dma_start(out=eye_t[:], in_=eye_d[:])
            nc.sync.dma_start(out=iotar_t[:], in_=iotar_d[:])
            nc.sync.dma_start(out=iotap_t[:], in_=iotap_d[:])
            nc.sync.dma_start(out=onesc_t[:], in_=onesc_d[:])
            nc.sync.dma_start(out=onesr_t[:], in_=onesr_d[:])
            nc.sync.dma_start(out=pscale_t[:], in_=pscale_d[:])
            nc.sync.dma_start(out=pbias_t[:], in_=pbias_d[:])
            nc.sync.dma_start(
                out=w_t[:].rearrange("p (c e) -> p c e", c=DC),
                in_=w_d[:].rearrange("(c p) e -> p c e", p=P),
            )

            if zero_fill:
                zt = sb.tile([P, 2048], f32, tag="zero")
                nc.vector.memset(zt[:], 0.0)
                flat = combine_d[:].rearrange("(n p) c -> n p (c)", p=P)
                # [NROW/128, 128, 64]; one DMA covers 32 n-slices
                nchunk = NROW // P // 32
                for i in range(nchunk):
                    nc.sync.dma_start(
                        out=flat[i * 32:(i + 1) * 32],
                        in_=zt[:].rearrange("p (a c) -> p a c", c=C),
                    )

            for g in range(gpc):
                # ================= gating GEMM =================
                lg_ps = ps.tile([P, SC * E], f32, tag="logits", bufs=2)
                for dc in range(DC):
                    xt = sb.tile([P, S], f32, tag="xt", bufs=4)
                    nc.sync.dma_start(
                        out=xt[:], in_=xT_d[g, dc * P:(dc + 1) * P, :])
                    for j in range(SC):
                        # start=True zeroes the whole 2KB PSUM bank, so only
                        # the very first matmul into this bank sets it.
                        nc.tensor.matmul(
                            out=lg_ps[:, j * E:(j + 1) * E],
                            lhsT=xt[:, j * P:(j + 1) * P],
                            rhs=w_t[:, dc * E:(dc + 1) * E],
                            start=(dc == 0 and j == 0),
                            stop=(dc == DC - 1 and j == SC - 1),
                        )

                # persistent per-group tiles
                oh_all = sb.tile([P, 2 * SC * C], f32, tag="oh", bufs=2)
                ef_all = sb.tile([P, 2 * SC], f32, tag="ef", bufs=2)
                val_all = sb.tile([P, 2 * SC], f32, tag="val", bufs=2)
                st_ps = ps.tile([E, 2], f32, tag="stats", bufs=2)

                # ================= per-chunk epilogue =================
                for j in range(SC):
                    lg = lg_ps[:, j * E:(j + 1) * E]
                    tok = slice(j * P, (j + 1) * P)

                    nz = sb.tile([P, E], f32, tag="nz", bufs=3)
                    nc.sync.dma_start(out=nz[:], in_=noise_d[g, tok, :])
                    noisy = sb.tile([P, E], f32, tag="noisy", bufs=3)
                    nc.vector.scalar_tensor_tensor(
                        out=noisy[:], in0=nz[:], scalar=1.0 / SIGMA_INV,
                        in1=lg, op0=OP.mult, op1=OP.add)

                    m8 = sb.tile([P, 8], f32, tag="m8", bufs=3)
                    nc.vector.max(out=m8[:], in_=noisy[:])
                    i8 = sb.tile([P, 8], u32, tag="i8", bufs=3)
                    nc.vector.max_index(out=i8[:], in_max=m8[:], in_values=noisy[:])

                    negm1 = sb.tile([P, 1], f32, tag="negm1", bufs=3)
                    nc.vector.tensor_scalar_mul(negm1[:], m8[:, 0:1], -1.0)

                    exn = sb.tile([P, E], f32, tag="exn", bufs=3)
                    zden = sb.tile([P, 1], f32, tag="zden", bufs=3)
                    nc.scalar.activation(
                        out=exn[:], in_=noisy[:], func=AF.Exp,
                        bias=negm1[:], scale=1.0, accum_out=zden[:])
                    # v0 = r = 1/Z  (gate of the top-1 expert)
                    nc.vector.reciprocal(val_all[:, j:j + 1], zden[:])

                    gn = sb.tile([P, E], f32, tag="gn", bufs=3)
                    nc.vector.tensor_scalar_mul(gn[:], exn[:], val_all[:, j:j + 1])
                    nc.sync.dma_start(out=gates_d[g, tok, :], in_=gn[:])

                    # v1 = exp(m2 - m1) * r  (gate of the top-2 expert)
                    v1e = sb.tile([P, 1], f32, tag="v1e", bufs=3)
                    nc.scalar.activation(
                        out=v1e[:], in_=m8[:, 1:2], func=AF.Exp,
                        bias=negm1[:], scale=1.0)
                    nc.vector.tensor_tensor(
                        out=val_all[:, SC + j:SC + j + 1], in0=v1e[:],
                        in1=val_all[:, j:j + 1], op=OP.mult)

                    # clean softmax -> importance partial sums
                    negcm = sb.tile([P, 1], f32, tag="negcm", bufs=3)
                    nc.vector.tensor_reduce(
                        out=negcm[:], in_=lg, axis=AX.X, op=OP.max, negate=True)
                    cex = sb.tile([P, E], f32, tag="cex", bufs=3)
                    czden = sb.tile([P, 1], f32, tag="czden", bufs=3)
                    nc.scalar.activation(
                        out=cex[:], in_=lg, func=AF.Exp,
                        bias=negcm[:], scale=1.0, accum_out=czden[:])
                    cr = sb.tile([P, 1], f32, tag="cr", bufs=3)
                    nc.vector.reciprocal(cr[:], czden[:])
                    nc.tensor.matmul(
                        out=st_ps[:, 0:1], lhsT=cex[:], rhs=cr[:],
                        start=(j == 0), stop=False)

                    # load-loss: sum_s erf((l - thr) * 64 / sqrt(2))
                    thrb = sb.tile([P, 1], f32, tag="thrb", bufs=3)
                    nc.vector.tensor_scalar_mul(thrb[:], m8[:, 1:2], -ERF_SCALE)
                    erf_t = sb.tile([P, E], f32, tag="erf", bufs=3)
                    nc.scalar.activation(
                        out=erf_t[:], in_=lg, func=erf_func,
                        bias=thrb[:], scale=ERF_SCALE)
                    nc.tensor.matmul(
                        out=st_ps[:, 1:2], lhsT=erf_t[:], rhs=onesc_t[:],
                        start=False, stop=(j == SC - 1))

                    # expert one-hots + f32 expert ids for the dispatcher
                    nc.vector.tensor_copy(out=ef_all[:, j:j + 1], in_=i8[:, 0:1])
                    nc.vector.tensor_copy(
                        out=ef_all[:, SC + j:SC + j + 1], in_=i8[:, 1:2])
                    nc.vector.tensor_scalar(
                        out=oh_all[:, j * C:(j + 1) * C], in0=iotar_t[:],
                        scalar1=ef_all[:, j:j + 1], scalar2=None,
                        op0=OP.is_equal)
                    nc.vector.tensor_scalar(
                        out=oh_all[:, (SC + j) * C:(SC + j + 1) * C],
                        in0=iotar_t[:],
                        scalar1=ef_all[:, SC + j:SC + j + 1], scalar2=None,
                        op0=OP.is_equal)

                # ============ dispatcher: positions + scatter ============
                # Running per-expert selection count lives in SBUF (cnt);
                # entering chunk t it holds the exclusive prefix.
                cnt = None
                for t in range(2 * SC):
                    j = t % SC
                    oh_t = oh_all[:, t * C:(t + 1) * C]
                    col_ps = ps.tile([1, E], f32, tag="col", bufs=1)
                    nc.tensor.matmul(
                        out=col_ps[:], lhsT=onesc_t[:], rhs=oh_t,
                        start=True, stop=True)
                    cs_ps = ps.tile([P, E], f32, tag="cs", bufs=2)
                    nc.tensor.matmul(
                        out=cs_ps[:], lhsT=u_t[:], rhs=oh_t,
                        start=True, stop=(t == 0))
                    if t > 0:
                        nc.tensor.matmul(
                            out=cs_ps[:], lhsT=onesr_t[:], rhs=cnt[:],
                            start=False, stop=True)
                    ncnt = sb.tile([1, E], f32, tag="cnt", bufs=3)
                    if t == 0:
                        nc.vector.tensor_copy(out=ncnt[:], in_=col_ps[:])
                    else:
                        nc.vector.tensor_tensor(
                            out=ncnt[:], in0=cnt[:], in1=col_ps[:], op=OP.add)
                    cnt = ncnt

                    tmp = sb.tile([P, E], f32, tag="postmp", bufs=3)
                    nc.vector.tensor_tensor(
                        out=tmp[:], in0=cs_ps[:], in1=oh_t, op=OP.mult)
                    pos1 = sb.tile([P, 1], f32, tag="pos1", bufs=3)
                    nc.vector.reduce_sum(out=pos1[:], in_=tmp[:], axis=AX.X)

                    # row index: e + 64*s + 65536*g ; dropped -> huge
                    idxf = sb.tile([P, 1], f32, tag="idxf", bufs=3)
                    nc.vector.scalar_tensor_tensor(
                        out=idxf[:], in0=ef_all[:, t:t + 1],
                        scalar=float(g * S * E + j * P * E),
                        in1=iotap_t[:], op0=OP.add, op1=OP.add)
                    dm = sb.tile([P, 1], f32, tag="dm", bufs=3)
                    nc.vector.tensor_scalar(
                        out=dm[:], in0=pos1[:], scalar1=float(C + 1),
                        scalar2=None, op0=OP.is_ge)
                    idxf2 = sb.tile([P, 1], f32, tag="idxf2", bufs=3)
                    # dropped rows get index 2^24 + real: > bounds_check but
                    # (idx*C) still far below 2^31 (no i32 overflow in DGE)
                    nc.vector.scalar_tensor_tensor(
                        out=idxf2[:], in0=dm[:], scalar=16777216.0, in1=idxf[:],
                        op0=OP.mult, op1=OP.add)
                    idx_i = sb.tile([P, 1], i32, tag="idxi", bufs=3)
                    nc.vector.tensor_copy(out=idx_i[:], in_=idxf2[:])

                    posc = sb.tile([P, 1], f32, tag="posc", bufs=3)
                    nc.vector.tensor_scalar(
                        out=posc[:], in0=pos1[:], scalar1=1.0, scalar2=None,
                        op0=OP.subtract)
                    rows = sb.tile([P, C], f32, tag="rows", bufs=3)
                    nc.vector.tensor_scalar(
                        out=rows[:], in0=iotar_t[:], scalar1=posc[:],
                        scalar2=val_all[:, t:t + 1], op0=OP.is_equal,
                        op1=OP.mult)

                    nc.gpsimd.indirect_dma_start(
                        out=combine_d[:],
                        out_offset=bass.IndirectOffsetOnAxis(
                            ap=idx_i[:, 0:1], axis=0),
                        in_=rows[:],
                        in_offset=None,
                        bounds_check=NROW - 1,
                        oob_is_err=False,
                    )

                # ================= losses =================
                st_sb = sb.tile([E, 2], f32, tag="st_sb", bufs=2)
                nc.vector.tensor_copy(out=st_sb[:], in_=st_ps[:])
                tr_ps = ps.tile([2, E], f32, tag="tr", bufs=1)
                nc.tensor.matmul(
                    out=tr_ps[:], lhsT=st_sb[:], rhs=eye_t[:],
                    start=True, stop=True)
                x2 = sb.tile([2, E], f32, tag="x2", bufs=2)
                nc.vector.tensor_scalar(
                    out=x2[:], in0=tr_ps[:], scalar1=pscale_t[:],
                    scalar2=pbias_t[:], op0=OP.mult, op1=OP.add)
                mean2 = sb.tile([2, 1], f32, tag="mean2", bufs=2)
                nc.vector.reduce_sum(out=mean2[:], in_=x2[:], axis=AX.X)
                nc.vector.tensor_scalar_mul(mean2[:], mean2[:], 1.0 / E)
                dev = sb.tile([2, E], f32, tag="dev", bufs=2)
                nc.vector.tensor_scalar(
                    out=dev[:], in0=x2[:], scalar1=mean2[:], scalar2=None,
                    op0=OP.subtract)
                var2 = sb.tile([2, 1], f32, tag="var2", bufs=2)
                nc.vector.scalar_tensor_tensor(
                    out=dev[:], in0=dev[:], scalar=1.0, in1=dev[:],
                    op0=OP.mult, op1=OP.mult, accum_out=var2[:])
                nc.vector.tensor_scalar_mul(var2[:], var2[:], 1.0 / E)
                imean = sb.tile([2, 1], f32, tag="imean", bufs=2)
                nc.vector.reciprocal(imean[:], mean2[:])
                loss2 = sb.tile([2, 1], f32, tag="loss2", bufs=2)
                nc.vector.tensor_tensor(
                    out=loss2[:], in0=imean[:], in1=imean[:], op=OP.mult)
                nc.vector.tensor_tensor(
                    out=loss2[:], in0=loss2[:], in1=var2[:], op=OP.mult)
                # transpose [2,1] -> [1,2], then aux = sum
                ltr_ps = ps.tile([1, 2], f32, tag="tr", bufs=1)
                nc.tensor.matmul(
                    out=ltr_ps[:], lhsT=loss2[:], rhs=eye_t[0:2, 0:2],
                    start=True, stop=True)
                lrow = sb.tile([1, 2], f32, tag="lrow", bufs=2)
                nc.vector.tensor_copy(out=lrow[:], in_=ltr_ps[:])
                auxv = sb.tile([1, 1], f32, tag="auxv", bufs=2)
                nc.vector.reduce_sum(out=auxv[:], in_=lrow[:], axis=AX.X)
                nc.sync.dma_start(out=imp_d[g:g + 1, :], in_=lrow[:, 0:1])
                nc.sync.dma_start(out=load_d[g:g + 1, :], in_=lrow[:, 1:2])
                nc.sync.dma_start(out=aux_d[g:g + 1, :], in_=auxv[:])

    nc.compile()
    return nc


_NC_CACHE = {}


def _get_nc(gpc, S, D, zero_fill=False):
    key = (gpc, S, D, zero_fill)
    if key not in _NC_CACHE:
        _NC_CACHE[key] = build_nc(gpc, S, D, zero_fill=zero_fill)
    return _NC_CACHE[key]


def make_in_maps(inputs, W, noise):
    G, S, D = inputs.shape
    gpc = G // N_CORES
    consts = make_consts(S, D)
    xs = np.asarray(inputs, dtype=np.float32).reshape(N_CORES, gpc, S, D)
    # host-side layout prep: [gpc, D, S] per core so the GEMM needs no
    # on-device transpose
    xT = np.ascontiguousarray(xs.transpose(0, 1, 3, 2))
    ns = np.ascontiguousarray(
        np.asarray(noise, dtype=np.float32).reshape(N_CORES, gpc, S, E))
    Wc = np.ascontiguousarray(np.asarray(W, dtype=np.float32))
    in_maps = []
    for c in range(N_CORES):
        im = {"xT": xT[c], "noiseg": ns[c], "Wg": Wc}
        im.update(consts)
        in_maps.append(im)
    return in_maps


def gather_outputs(results, G, S):
    gpc = G // N_CORES
    combine = np.concatenate(
        [r["combine"].reshape(gpc, S, E, C) for r in results], axis=0)
    gates = np.concatenate(
        [r["gates_noisy"].reshape(gpc, S, E) for r in results], axis=0)
    imp = np.concatenate([r["imp_out"].reshape(gpc) for r in results])
    load = np.concatenate([r["load_out"].reshape(gpc) for r in results])
    aux = np.concatenate([r["aux_out"].reshape(gpc) for r in results])
    return combine, aux, imp, load, gates


def run(inputs, W, noise, trace=False, zero_fill=False):
    m = _imports()
    G, S, D = inputs.shape
    gpc = G // N_CORES
    nc = _get_nc(gpc, S, D, zero_fill=zero_fill)
    in_maps = make_in_maps(inputs, W, noise)
    res = m["run_bass_kernel_spmd"](
        nc, in_maps, list(range(N_CORES)), trace=trace)
    outs = gather_outputs(res.results, G, S)
    return outs, res


def kernel(inputs, W, noise):
    outs, _ = run(np.asarray(inputs), np.asarray(W), np.asarray(noise))
    return outs


# revision 25
# speedup vs baseline: 1.2436x; 1.2436x over previous
"""Trainium2 Bass kernel for the noisy top-2-experts MoE router.

Contract: kernel(**inputs) takes the FULL inputs
    inputs: [G=16, S=1024, D=4096] f32
    W:      [D=4096, E=64] f32
    noise:  [G=16, S=1024, E=64] f32
and returns the full reference outputs
    (combine [G,S,E,C], aux_loss [G], importance_loss [G], load_loss [G],
     gates_noisy [G,S,E])

Sharding: group axis G across 8 NeuronCores (2 groups per core); the small
gating weight W is replicated.  Inputs are pre-transposed per group on the
host to [D, S] so the gating GEMM contracts over the partition axis without
any on-device transpose.

Device algorithm per group:
  1. logits = x @ W via PE, accumulating [128 tok, 64 exp] slices packed as
     one [128, 512] PSUM bank per group.
  2. Per 128-token chunk: noisy logits, Max8/MaxIndex top-2, exp/softmax on
     ACT (accum_out gives the softmax denominator), importance & load-loss
     partial sums accumulated with tiny PE matmuls, and expert one-hots for
     the dispatcher.
  3. Dispatcher positions via cumulative-sum-as-matmul: an upper-triangular
     constant U gives the within-chunk inclusive cumsum; a K=1 rank-1 matmul
     adds the running per-expert prefix.
  4. The huge sparse combine tensor is never densified: each selection's
     64-float capacity row is scattered with indirect DMA into the
     pre-zeroed DRAM output (out-of-bounds row index == dropped token).
"""

import math

import numpy as np

F32 = None  # filled lazily in _imports
_IMPORTS = {}


def _imports():
    global F32
    if _IMPORTS:
        return _IMPORTS
    import concourse.bacc as bacc
    import concourse.bass as bass
    import concourse.mybir as mybir
    import concourse.tile as tile
    from concourse.bass_utils import run_bass_kernel_spmd

    _IMPORTS.update(
        bacc=bacc, bass=bass, mybir=mybir, tile=tile,
        run_bass_kernel_spmd=run_bass_kernel_spmd,
    )
    F32 = mybir.dt.float32
    return _IMPORTS


N_CORES = 8
E = 64          # experts
C = 64          # capacity
TOPK = 2
SIGMA_INV = float(E)                       # 1 / (NOISE_STD / NUM_EXPERTS)
ERF_SCALE = float(E) / math.sqrt(2.0)      # z -> erf argument scale
P = 128


def make_consts(S, D):
    """Constant tensors shipped to every core."""
    DC = D // P
    u128 = np.triu(np.ones((P, P), dtype=np.float32))          # U[j,i]=1 if j<=i
    eye64 = np.eye(E, dtype=np.float32)
    iota_row = np.tile(np.arange(C, dtype=np.float32), (P, 1))  # [128, 64]
    iotap = (np.arange(P, dtype=np.float32) * E)[:, None]       # [128, 1] p*64
    ones_col = np.ones((P, 1), dtype=np.float32)
    ones_row = np.ones((1, P), dtype=np.float32)
    post_scale = np.array([[1.0], [0.5 / S]], dtype=np.float32)
    post_bias = np.array([[0.0], [0.5]], dtype=np.float32)
    return {
        "u128": u128, "eye64": eye64, "iota_row": iota_row, "iotap": iotap,
        "ones_col": ones_col, "ones_row": ones_row,
        "post_scale": post_scale, "post_bias": post_bias,
    }


def build_nc(gpc, S, D, zero_fill=False, erf_func=None):
    """Build + compile the per-core Bass program.

    gpc: groups per core; S: tokens per group; D: model dim.
    zero_fill: also zero the combine output from the device (fallback when
    pre-zeroed ExternalOutput buffers can't be relied on).
    """
    m = _imports()
    bacc, bass, mybir, tile = m["bacc"], m["bass"], m["mybir"], m["tile"]
    AF = mybir.ActivationFunctionType
    if erf_func is None:
        erf_func = AF.Erf  # CoreSim lacks Erf; tests may substitute Tanh
    OP = mybir.AluOpType
    AX = mybir.AxisListType
    f32 = mybir.dt.float32
    i32 = mybir.dt.int32
    u32 = mybir.dt.uint32

    SC = S // P     # token chunks per group
    DC = D // P     # contraction chunks
    NROW = gpc * S * E   # combine rows per core

    nc = bacc.Bacc(
        "TRN2", target_bir_lowering=False, debug=False,
        enable_asserts=False,
    )

    xT_d = nc.dram_tensor("xT", [gpc, D, S], f32, kind="ExternalInput")
    noise_d = nc.dram_tensor("noiseg", [gpc, S, E], f32, kind="ExternalInput")
    w_d = nc.dram_tensor("Wg", [D, E], f32, kind="ExternalInput")
    u_d = nc.dram_tensor("u128", [P, P], f32, kind="ExternalInput")
    eye_d = nc.dram_tensor("eye64", [E, E], f32, kind="ExternalInput")
    iotar_d = nc.dram_tensor("iota_row", [P, C], f32, kind="ExternalInput")
    iotap_d = nc.dram_tensor("iotap", [P, 1], f32, kind="ExternalInput")
    onesc_d = nc.dram_tensor("ones_col", [P, 1], f32, kind="ExternalInput")
    onesr_d = nc.dram_tensor("ones_row", [1, P], f32, kind="ExternalInput")
    pscale_d = nc.dram_tensor("post_scale", [2, 1], f32, kind="ExternalInput")
    pbias_d = nc.dram_tensor("post_bias", [2, 1], f32, kind="ExternalInput")

    combine_d = nc.dram_tensor("combine", [NROW, C], f32, kind="ExternalOutput")
    gates_d = nc.dram_tensor("gates_noisy", [gpc, S, E], f32, kind="ExternalOutput")
    imp_d = nc.dram_tensor("imp_out", [gpc, 1], f32, kind="ExternalOutput")
    load_d = nc.dram_tensor("load_out", [gpc, 1], f32, kind="ExternalOutput")
    aux_d = nc.dram_tensor("aux_out", [gpc, 1], f32, kind="ExternalOutput")

    with tile.TileContext(nc) as tc:
        with tc.tile_pool(name="sb", bufs=1) as sb, \
             tc.tile_pool(name="ps", bufs=1, space="PSUM") as ps:

            # ---- persistent constants ----
            u_t = sb.tile([P, P], f32, tag="u128")
            eye_t = sb.tile([E, E], f32, tag="eye64")
            iotar_t = sb.tile([P, C], f32, tag="iotar")
            iotap_t = sb.tile([P, 1], f32, tag="iotap")
            onesc_t = sb.tile([P, 1], f32, tag="onesc")
            onesr_t = sb.tile([1, P], f32, tag="onesr")
            pscale_t = sb.tile([2, 1], f32, tag="pscale")
            pbias_t = sb.tile([2, 1], f32, tag="pbias")
            w_t = sb.tile([P, DC * E], f32, tag="w")
            nc.sync.dma_start(out=u_t[:], in_=u_d[:])
            nc.sync.dma_start(out=eye_t[:], in_=eye_d[:])
            nc.sync.dma_start(out=iotar_t[:], in_=iotar_d[:])
            nc.sync.dma_start(out=iotap_t[:], in_=iotap_d[:])
            nc.sync.dma_start(out=onesc_t[:], in_=onesc_d[:])
            nc.sync.dma_start(out=onesr_t[:], in_=onesr_d[:])
            nc.sync.dma_start(out=pscale_t[:], in_=pscale_d[:])
            nc.sync.dma_start(out=pbias_t[:], in_=pbias_d[:])
            nc.sync.dma_start(
                out=w_t[:].rearrange("p (c e) -> p c e", c=DC),
                in_=w_d[:].rearrange("(c p) e -> p c e", p=P),
            )

            if zero_fill:
                zt = sb.tile([P, 2048], f32, tag="zero")
                nc.vector.memset(zt[:], 0.0)
                flat = combine_d[:].rearrange("(n p) c -> n p (c)", p=P)
                # [NROW/128, 128, 64]; one DMA covers 32 n-slices
                nchunk = NROW // P // 32
                for i in range(nchunk):
                    nc.sync.dma_start(
                        out=flat[i * 32:(i + 1) * 32],
                        in_=zt[:].rearrange("p (a c) -> p a c", c=C),
                    )

            for g in range(gpc):
                # ================= gating GEMM =================
                # W is the (cheap, 64-column) stationary operand; tokens
                # stream as the moving operand.  Output is logits^T [E, S]
                # (fp32 weight loads are slow and per-matmul; this cuts the
                # loaded columns by ~20x vs token-stationary).
                lgT_ps = ps.tile([E, S], f32, tag="lgT", bufs=1)
                for dc in range(DC):
                    xt = sb.tile([P, S], f32, tag="xt", bufs=4)
                    nc.sync.dma_start(
                        out=xt[:], in_=xT_d[g, dc * P:(dc + 1) * P, :])
                    for s0 in range(0, S, 512):  # fp32 moving-operand limit
                        sw = min(512, S - s0)
                        nc.tensor.matmul(
                            out=lgT_ps[:, s0:s0 + sw],
                            lhsT=w_t[:, dc * E:(dc + 1) * E],
                            rhs=xt[:, s0:s0 + sw],
                            start=(dc == 0),
                            stop=(dc == DC - 1),
                        )
                lgT_sb = sb.tile([E, S], f32, tag="lgTsb", bufs=2)
                nc.scalar.copy(out=lgT_sb[:], in_=lgT_ps[:])

                # persistent per-group tiles
                lg_all = sb.tile([P, SC * E], f32, tag="lg", bufs=2)
                oh_all = sb.tile([P, 2 * SC * C], f32, tag="oh", bufs=2)
                ef_all = sb.tile([P, 2 * SC], f32, tag="ef", bufs=2)
                val_all = sb.tile([P, 2 * SC], f32, tag="val", bufs=2)
                thr_all = sb.tile([P, SC], f32, tag="thr", bufs=2)
                st_ps = ps.tile([E, 2], f32, tag="stats", bufs=2)

                # transpose logits back to [token, expert] chunks via the
                # DVE 32x32 block-transpose unit (PE is_transpose matmuls
                # interleaved with fp32 HI/LO matmul pairs hang the PE)
                for j in range(SC):
                    for sbk in range(P // 32):
                        for ebk in range(E // 32):
                            nc.vector.transpose(
                                out=lg_all[32 * sbk:32 * (sbk + 1),
                                           j * E + 32 * ebk:
                                           j * E + 32 * (ebk + 1)],
                                in_=lgT_sb[32 * ebk:32 * (ebk + 1),
                                           j * P + 32 * sbk:
                                           j * P + 32 * (sbk + 1)])

                # ============ per-chunk epilogue (exp phase) ============
                for j in range(SC):
                    lg = lg_all[:, j * E:(j + 1) * E]
                    tok = slice(j * P, (j + 1) * P)

                    nz = sb.tile([P, E], f32, tag="nz", bufs=3)
                    nc.sync.dma_start(out=nz[:], in_=noise_d[g, tok, :])
                    noisy = sb.tile([P, E], f32, tag="noisy", bufs=3)
                    nc.vector.scalar_tensor_tensor(
                        out=noisy[:], in0=nz[:], scalar=1.0 / SIGMA_INV,
                        in1=lg, op0=OP.mult, op1=OP.add)

                    m8 = sb.tile([P, 8], f32, tag="m8", bufs=3)
                    nc.vector.max(out=m8[:], in_=noisy[:])
                    i8 = sb.tile([P, 8], u32, tag="i8", bufs=3)
                    nc.vector.max_index(out=i8[:], in_max=m8[:], in_values=noisy[:])

                    negm1 = sb.tile([P, 1], f32, tag="negm1", bufs=3)
                    nc.vector.tensor_scalar_mul(negm1[:], m8[:, 0:1], -1.0)

                    exn = sb.tile([P, E], f32, tag="exn", bufs=3)
                    zden = sb.tile([P, 1], f32, tag="zden", bufs=3)
                    nc.scalar.activation(
                        out=exn[:], in_=noisy[:], func=AF.Exp,
                        bias=negm1[:], scale=1.0, accum_out=zden[:])
                    # v0 = r = 1/Z  (gate of the top-1 expert)
                    nc.vector.reciprocal(val_all[:, j:j + 1], zden[:])

                    gn = sb.tile([P, E], f32, tag="gn", bufs=3)
                    nc.vector.tensor_scalar_mul(gn[:], exn[:], val_all[:, j:j + 1])
                    nc.sync.dma_start(out=gates_d[g, tok, :], in_=gn[:])

                    # v1 = exp(m2 - m1) * r  (gate of the top-2 expert)
                    v1e = sb.tile([P, 1], f32, tag="v1e", bufs=3)
                    nc.scalar.activation(
                        out=v1e[:], in_=m8[:, 1:2], func=AF.Exp,
                        bias=negm1[:], scale=1.0)
                    nc.vector.tensor_tensor(
                        out=val_all[:, SC + j:SC + j + 1], in0=v1e[:],
                        in1=val_all[:, j:j + 1], op=OP.mult)

                    # clean softmax -> importance partial sums
                    negcm = sb.tile([P, 1], f32, tag="negcm", bufs=3)
                    nc.vector.tensor_reduce(
                        out=negcm[:], in_=lg, axis=AX.X, op=OP.max, negate=True)
                    cex = sb.tile([P, E], f32, tag="cex", bufs=3)
                    czden = sb.tile([P, 1], f32, tag="czden", bufs=3)
                    nc.scalar.activation(
                        out=cex[:], in_=lg, func=AF.Exp,
                        bias=negcm[:], scale=1.0, accum_out=czden[:])
                    cr = sb.tile([P, 1], f32, tag="cr", bufs=3)
                    nc.vector.reciprocal(cr[:], czden[:])
                    nc.tensor.matmul(
                        out=st_ps[:, 0:1], lhsT=cex[:], rhs=cr[:],
                        start=(j == 0), stop=False)

                    # threshold bias for the erf phase below
                    nc.vector.tensor_scalar_mul(
                        thr_all[:, j:j + 1], m8[:, 1:2], -ERF_SCALE)

                    # expert one-hots + f32 expert ids for the dispatcher
                    nc.vector.tensor_copy(out=ef_all[:, j:j + 1], in_=i8[:, 0:1])
                    nc.vector.tensor_copy(
                        out=ef_all[:, SC + j:SC + j + 1], in_=i8[:, 1:2])
                    nc.vector.tensor_scalar(
                        out=oh_all[:, j * C:(j + 1) * C], in0=iotar_t[:],
                        scalar1=ef_all[:, j:j + 1], scalar2=None,
                        op0=OP.is_equal)
                    nc.vector.tensor_scalar(
                        out=oh_all[:, (SC + j) * C:(SC + j + 1) * C],
                        in0=iotar_t[:],
                        scalar1=ef_all[:, SC + j:SC + j + 1], scalar2=None,
                        op0=OP.is_equal)

                # ============ erf phase (one ACT table switch per group) ====
                # load-loss: sum_s erf((l - thr) * 64 / sqrt(2))
                for j in range(SC):
                    erf_t = sb.tile([P, E], f32, tag="erf", bufs=3)
                    nc.scalar.activation(
                        out=erf_t[:], in_=lg_all[:, j * E:(j + 1) * E],
                        func=erf_func,
                        bias=thr_all[:, j:j + 1], scale=ERF_SCALE)
                    nc.tensor.matmul(
                        out=st_ps[:, 1:2], lhsT=erf_t[:], rhs=onesc_t[:],
                        start=False, stop=(j == SC - 1))

                # ============ dispatcher: positions + scatter ============
                # Running per-expert selection count lives in SBUF (cnt);
                # entering chunk t it holds the exclusive prefix.
                cnt = None
                for t in range(2 * SC):
                    j = t % SC
                    oh_t = oh_all[:, t * C:(t + 1) * C]
                    col_ps = ps.tile([1, E], f32, tag="col", bufs=1)
                    nc.tensor.matmul(
                        out=col_ps[:], lhsT=onesc_t[:], rhs=oh_t,
                        start=True, stop=True)
                    cs_ps = ps.tile([P, E], f32, tag="cs", bufs=2)
                    nc.tensor.matmul(
                        out=cs_ps[:], lhsT=u_t[:], rhs=oh_t,
                        start=True, stop=(t == 0))
                    if t > 0:
                        nc.tensor.matmul(
                            out=cs_ps[:], lhsT=onesr_t[:], rhs=cnt[:],
                            start=False, stop=True)
                    ncnt = sb.tile([1, E], f32, tag="cnt", bufs=3)
                    if t == 0:
                        nc.vector.tensor_copy(out=ncnt[:], in_=col_ps[:])
                    else:
                        nc.vector.tensor_tensor(
                            out=ncnt[:], in0=cnt[:], in1=col_ps[:], op=OP.add)
                    cnt = ncnt

                    tmp = sb.tile([P, E], f32, tag="postmp", bufs=3)
                    nc.vector.tensor_tensor(
                        out=tmp[:], in0=cs_ps[:], in1=oh_t, op=OP.mult)
                    pos1 = sb.tile([P, 1], f32, tag="pos1", bufs=3)
                    nc.vector.reduce_sum(out=pos1[:], in_=tmp[:], axis=AX.X)

                    # row index: e + 64*s + 65536*g ; dropped -> huge
                    idxf = sb.tile([P, 1], f32, tag="idxf", bufs=3)
                    nc.vector.scalar_tensor_tensor(
                        out=idxf[:], in0=ef_all[:, t:t + 1],
                        scalar=float(g * S * E + j * P * E),
                        in1=iotap_t[:], op0=OP.add, op1=OP.add)
                    dm = sb.tile([P, 1], f32, tag="dm", bufs=3)
                    nc.vector.tensor_scalar(
                        out=dm[:], in0=pos1[:], scalar1=float(C + 1),
                        scalar2=None, op0=OP.is_ge)
                    idxf2 = sb.tile([P, 1], f32, tag="idxf2", bufs=3)
                    # dropped rows get index 2^24 + real: > bounds_check but
                    # (idx*C) still far below 2^31 (no i32 overflow in DGE)
                    nc.vector.scalar_tensor_tensor(
                        out=idxf2[:], in0=dm[:], scalar=16777216.0, in1=idxf[:],
                        op0=OP.mult, op1=OP.add)
                    idx_i = sb.tile([P, 1], i32, tag="idxi", bufs=3)
                    nc.vector.tensor_copy(out=idx_i[:], in_=idxf2[:])

                    posc = sb.tile([P, 1], f32, tag="posc", bufs=3)
                    nc.vector.tensor_scalar(
                        out=posc[:], in0=pos1[:], scalar1=1.0, scalar2=None,
                        op0=OP.subtract)
                    rows = sb.tile([P, C], f32, tag="rows", bufs=3)
                    nc.vector.tensor_scalar(
                        out=rows[:], in0=iotar_t[:], scalar1=posc[:],
                        scalar2=val_all[:, t:t + 1], op0=OP.is_equal,
                        op1=OP.mult)

                    nc.gpsimd.indirect_dma_start(
                        out=combine_d[:],
                        out_offset=bass.IndirectOffsetOnAxis(
                            ap=idx_i[:, 0:1], axis=0),
                        in_=rows[:],
                        in_offset=None,
                        bounds_check=NROW - 1,
                        oob_is_err=False,
                    )

                # ================= losses =================
                st_sb = sb.tile([E, 2], f32, tag="st_sb", bufs=2)
                nc.vector.tensor_copy(out=st_sb[:], in_=st_ps[:])
                tr_ps = ps.tile([2, E], f32, tag="tr", bufs=1)
                nc.tensor.matmul(
                    out=tr_ps[:], lhsT=st_sb[:], rhs=eye_t[:],
                    start=True, stop=True)
                x2 = sb.tile([2, E], f32, tag="x2", bufs=2)
                nc.vector.tensor_scalar(
                    out=x2[:], in0=tr_ps[:], scalar1=pscale_t[:],
                    scalar2=pbias_t[:], op0=OP.mult, op1=OP.add)
                mean2 = sb.tile([2, 1], f32, tag="mean2", bufs=2)
                nc.vector.reduce_sum(out=mean2[:], in_=x2[:], axis=AX.X)
                nc.vector.tensor_scalar_mul(mean2[:], mean2[:], 1.0 / E)
                dev = sb.tile([2, E], f32, tag="dev", bufs=2)
                nc.vector.tensor_scalar(
                    out=dev[:], in0=x2[:], scalar1=mean2[:], scalar2=None,
                    op0=OP.subtract)
                var2 = sb.tile([2, 1], f32, tag="var2", bufs=2)
                nc.vector.scalar_tensor_tensor(
                    out=dev[:], in0=dev[:], scalar=1.0, in1=dev[:],
                    op0=OP.mult, op1=OP.mult, accum_out=var2[:])
                nc.vector.tensor_scalar_mul(var2[:], var2[:], 1.0 / E)
                imean = sb.tile([2, 1], f32, tag="imean", bufs=2)
                nc.vector.reciprocal(imean[:], mean2[:])
                loss2 = sb.tile([2, 1], f32, tag="loss2", bufs=2)
                nc.vector.tensor_tensor(
                    out=loss2[:], in0=imean[:], in1=imean[:], op=OP.mult)
                nc.vector.tensor_tensor(
                    out=loss2[:], in0=loss2[:], in1=var2[:], op=OP.mult)
                # transpose [2,1] -> [1,2], then aux = sum
                ltr_ps = ps.tile([1, 2], f32, tag="tr", bufs=1)
                nc.tensor.matmul(
                    out=ltr_ps[:], lhsT=loss2[:], rhs=eye_t[0:2, 0:2],
                    start=True, stop=True)
                lrow = sb.tile([1, 2], f32, tag="lrow", bufs=2)
                nc.vector.tensor_copy(out=lrow[:], in_=ltr_ps[:])
                auxv = sb.tile([1, 1], f32, tag="auxv", bufs=2)
                nc.vector.reduce_sum(out=auxv[:], in_=lrow[:], axis=AX.X)
                nc.sync.dma_start(out=imp_d[g:g + 1, :], in_=lrow[:, 0:1])
                nc.sync.dma_start(out=load_d[g:g + 1, :], in_=lrow[:, 1:2])
                nc.sync.dma_start(out=aux_d[g:g + 1, :], in_=auxv[:])

    nc.compile()
    return nc


_NC_CACHE = {}


def _get_nc(gpc, S, D, zero_fill=False):
    key = (gpc, S, D, zero_fill)
    if key not in _NC_CACHE:
        _NC_CACHE[key] = build_nc(gpc, S, D, zero_fill=zero_fill)
    return _NC_CACHE[key]


def make_in_maps(inputs, W, noise):
    G, S, D = inputs.shape
    gpc = G // N_CORES
    consts = make_consts(S, D)
    xs = np.asarray(inputs, dtype=np.float32).reshape(N_CORES, gpc, S, D)
    # host-side layout prep: [gpc, D, S] per core so the GEMM needs no
    # on-device transpose
    xT = np.ascontiguousarray(xs.transpose(0, 1, 3, 2))
    ns = np.ascontiguousarray(
        np.asarray(noise, dtype=np.float32).reshape(N_CORES, gpc, S, E))
    Wc = np.ascontiguousarray(np.asarray(W, dtype=np.float32))
    in_maps = []
    for c in range(N_CORES):
        im = {"xT": xT[c], "noiseg": ns[c], "Wg": Wc}
        im.update(consts)
        in_maps.append(im)
    return in_maps


def gather_outputs(results, G, S):
    gpc = G // N_CORES
    combine = np.concatenate(
        [r["combine"].reshape(gpc, S, E, C) for r in results], axis=0)
    gates = np.concatenate(
        [r["gates_noisy"].reshape(gpc, S, E) for r in results], axis=0)
    imp = np.concatenate([r["imp_out"].reshape(gpc) for r in results])
    load = np.concatenate([r["load_out"].reshape(gpc) for r in results])
    aux = np.concatenate([r["aux_out"].reshape(gpc) for r in results])
    return combine, aux, imp, load, gates


def run(inputs, W, noise, trace=False, zero_fill=False):
    m = _imports()
    G, S, D = inputs.shape
    gpc = G // N_CORES
    nc = _get_nc(gpc, S, D, zero_fill=zero_fill)
    in_maps = make_in_maps(inputs, W, noise)
    res = m["run_bass_kernel_spmd"](
        nc, in_maps, list(range(N_CORES)), trace=trace)
    outs = gather_outputs(res.results, G, S)
    return outs, res


def kernel(inputs, W, noise):
    outs, _ = run(np.asarray(inputs), np.asarray(W), np.asarray(noise))
    return outs


# revision 37
# speedup vs baseline: 1.4581x; 1.1725x over previous
"""Trainium2 Bass kernel for the noisy top-2-experts MoE router.

Contract: kernel(**inputs) takes the FULL inputs
    inputs: [G=16, S=1024, D=4096] f32
    W:      [D=4096, E=64] f32
    noise:  [G=16, S=1024, E=64] f32
and returns the full reference outputs
    (combine [G,S,E,C], aux_loss [G], importance_loss [G], load_loss [G],
     gates_noisy [G,S,E])

Sharding: group axis G across 8 NeuronCores (2 groups per core); the small
gating weight W is replicated.  Inputs are pre-transposed per group on the
host to [D, S] so the gating GEMM contracts over the partition axis without
any on-device transpose.

Device algorithm per group:
  1. logits = x @ W via PE, accumulating [128 tok, 64 exp] slices packed as
     one [128, 512] PSUM bank per group.
  2. Per 128-token chunk: noisy logits, Max8/MaxIndex top-2, exp/softmax on
     ACT (accum_out gives the softmax denominator), importance & load-loss
     partial sums accumulated with tiny PE matmuls, and expert one-hots for
     the dispatcher.
  3. Dispatcher positions via cumulative-sum-as-matmul: an upper-triangular
     constant U gives the within-chunk inclusive cumsum; a K=1 rank-1 matmul
     adds the running per-expert prefix.
  4. The huge sparse combine tensor is never densified: each selection's
     64-float capacity row is scattered with indirect DMA into the
     pre-zeroed DRAM output (out-of-bounds row index == dropped token).
"""

import math

import numpy as np

F32 = None  # filled lazily in _imports
_IMPORTS = {}


def _imports():
    global F32
    if _IMPORTS:
        return _IMPORTS
    import concourse.bacc as bacc
    import concourse.bass as bass
    import concourse.mybir as mybir
    import concourse.tile as tile
    from concourse.bass_utils import run_bass_kernel_spmd

    _IMPORTS.update(
        bacc=bacc, bass=bass, mybir=mybir, tile=tile,
        run_bass_kernel_spmd=run_bass_kernel_spmd,
    )
    F32 = mybir.dt.float32
    return _IMPORTS


N_CORES = 8
E = 64          # experts
C = 64          # capacity
TOPK = 2
SIGMA_INV = float(E)                       # 1 / (NOISE_STD / NUM_EXPERTS)
ERF_SCALE = float(E) / math.sqrt(2.0)      # z -> erf argument scale
P = 128


def make_consts(S, D):
    """Constant tensors shipped to every core."""
    import ml_dtypes
    bf = ml_dtypes.bfloat16
    u128 = np.triu(np.ones((P, P), dtype=np.float32))          # U[j,i]=1 if j<=i
    eye64 = np.eye(E, dtype=np.float32)
    iota_row = np.tile(np.arange(C, dtype=np.float32), (P, 1))  # [128, 64]
    iotap = (np.arange(P, dtype=np.float32) * E)[:, None]       # [128, 1] p*64
    ones_col = np.ones((P, 1), dtype=np.float32)
    ones_colb = np.ones((P, 1), dtype=bf)
    ones_row = np.ones((1, P), dtype=np.float32)
    post_scale = np.array([[1.0], [0.5 / S]], dtype=np.float32)
    post_bias = np.array([[0.0], [0.5]], dtype=np.float32)
    return {
        "u128": u128, "eye64": eye64, "iota_row": iota_row, "iotap": iotap,
        "ones_col": ones_col, "ones_colb": ones_colb, "ones_row": ones_row,
        "post_scale": post_scale, "post_bias": post_bias,
    }


def build_nc(gpc, S, D, zero_fill=False, erf_func=None):
    """Build + compile the per-core Bass program.

    gpc: groups per core; S: tokens per group; D: model dim.
    zero_fill: also zero the combine output from the device (fallback when
    pre-zeroed ExternalOutput buffers can't be relied on).
    """
    m = _imports()
    bacc, bass, mybir, tile = m["bacc"], m["bass"], m["mybir"], m["tile"]
    AF = mybir.ActivationFunctionType
    if erf_func is None:
        erf_func = AF.Erf  # CoreSim lacks Erf; tests may substitute Tanh
    OP = mybir.AluOpType
    AX = mybir.AxisListType
    f32 = mybir.dt.float32
    i32 = mybir.dt.int32
    u32 = mybir.dt.uint32

    SC = S // P     # token chunks per group
    DC = D // P     # contraction chunks
    NROW = gpc * S * E   # combine rows per core

    bf = mybir.dt.bfloat16

    nc = bacc.Bacc(
        "TRN2", target_bir_lowering=False, debug=False,
        enable_asserts=False,
    )

    xh_d = nc.dram_tensor("xh", [gpc, D, S], bf, kind="ExternalInput")
    xl_d = nc.dram_tensor("xl", [gpc, D, S], bf, kind="ExternalInput")
    noise_d = nc.dram_tensor("noiseg", [gpc, S, E], f32, kind="ExternalInput")
    wh_d = nc.dram_tensor("Wh", [D, E], bf, kind="ExternalInput")
    wl_d = nc.dram_tensor("Wl", [D, E], bf, kind="ExternalInput")
    u_d = nc.dram_tensor("u128", [P, P], f32, kind="ExternalInput")
    eye_d = nc.dram_tensor("eye64", [E, E], f32, kind="ExternalInput")
    iotar_d = nc.dram_tensor("iota_row", [P, C], f32, kind="ExternalInput")
    iotap_d = nc.dram_tensor("iotap", [P, 1], f32, kind="ExternalInput")
    onesc_d = nc.dram_tensor("ones_col", [P, 1], f32, kind="ExternalInput")
    onescb_d = nc.dram_tensor("ones_colb", [P, 1], bf, kind="ExternalInput")
    onesr_d = nc.dram_tensor("ones_row", [1, P], f32, kind="ExternalInput")
    pscale_d = nc.dram_tensor("post_scale", [2, 1], f32, kind="ExternalInput")
    pbias_d = nc.dram_tensor("post_bias", [2, 1], f32, kind="ExternalInput")

    combine_d = nc.dram_tensor("combine", [NROW, C], f32, kind="ExternalOutput")
    gates_d = nc.dram_tensor("gates_noisy", [gpc, S, E], f32, kind="ExternalOutput")
    imp_d = nc.dram_tensor("imp_out", [gpc, 1], f32, kind="ExternalOutput")
    load_d = nc.dram_tensor("load_out", [gpc, 1], f32, kind="ExternalOutput")
    aux_d = nc.dram_tensor("aux_out", [gpc, 1], f32, kind="ExternalOutput")

    with tile.TileContext(nc) as tc:
        with tc.tile_pool(name="sb", bufs=1) as sb, \
             tc.tile_pool(name="ps", bufs=1, space="PSUM") as ps:

            # ---- persistent constants ----
            u_t = sb.tile([P, P], f32, tag="u128")
            eye_t = sb.tile([E, E], f32, tag="eye64")
            iotar_t = sb.tile([P, C], f32, tag="iotar")
            iotap_t = sb.tile([P, 1], f32, tag="iotap")
            onesc_t = sb.tile([P, 1], f32, tag="onesc")
            onescb_t = sb.tile([P, 1], bf, tag="onescb")
            onesr_t = sb.tile([1, P], f32, tag="onesr")
            pscale_t = sb.tile([2, 1], f32, tag="pscale")
            pbias_t = sb.tile([2, 1], f32, tag="pbias")
            wh_t = sb.tile([P, DC * E], bf, tag="wh")
            wl_t = sb.tile([P, DC * E], bf, tag="wl")
            nc.sync.dma_start(out=u_t[:], in_=u_d[:])
            nc.sync.dma_start(out=eye_t[:], in_=eye_d[:])
            nc.sync.dma_start(out=iotar_t[:], in_=iotar_d[:])
            nc.sync.dma_start(out=iotap_t[:], in_=iotap_d[:])
            nc.sync.dma_start(out=onesc_t[:], in_=onesc_d[:])
            nc.sync.dma_start(out=onescb_t[:], in_=onescb_d[:])
            nc.sync.dma_start(out=onesr_t[:], in_=onesr_d[:])
            nc.sync.dma_start(out=pscale_t[:], in_=pscale_d[:])
            nc.sync.dma_start(out=pbias_t[:], in_=pbias_d[:])
            nc.sync.dma_start(
                out=wh_t[:].rearrange("p (c e) -> p c e", c=DC),
                in_=wh_d[:].rearrange("(c p) e -> p c e", p=P),
            )
            nc.sync.dma_start(
                out=wl_t[:].rearrange("p (c e) -> p c e", c=DC),
                in_=wl_d[:].rearrange("(c p) e -> p c e", p=P),
            )

            if zero_fill:
                zt = sb.tile([P, 2048], f32, tag="zero")
                nc.vector.memset(zt[:], 0.0)
                flat = combine_d[:].rearrange("(n p) c -> n p (c)", p=P)
                # [NROW/128, 128, 64]; one DMA covers 32 n-slices
                nchunk = NROW // P // 32
                for i in range(nchunk):
                    nc.sync.dma_start(
                        out=flat[i * 32:(i + 1) * 32],
                        in_=zt[:].rearrange("p (a c) -> p a c", c=C),
                    )

            for g in range(gpc):
                # ================= gating GEMM =================
                # W is the (cheap, 64-column) stationary operand; tokens
                # stream as the moving operand.  Output is logits^T [E, S]
                # (fp32 weight loads are slow and per-matmul; this cuts the
                # loaded columns by ~20x vs token-stationary).
                # split-precision: logits = xh@Wh + xh@Wl + xl@Wh
                # (bf16 streams 1 col/cycle on PE vs fp32's effective 4)
                lgT_ps = ps.tile([E, S], f32, tag="lgT", bufs=1)
                for dc in range(DC):
                    wsl = slice(dc * E, (dc + 1) * E)
                    xh_t = sb.tile([P, S], bf, tag="xh", bufs=4)
                    xl_t = sb.tile([P, S], bf, tag="xl", bufs=4)
                    nc.sync.dma_start(
                        out=xh_t[:], in_=xh_d[g, dc * P:(dc + 1) * P, :])
                    nc.sync.dma_start(
                        out=xl_t[:], in_=xl_d[g, dc * P:(dc + 1) * P, :])
                    for s0 in range(0, S, 512):  # one PSUM bank per matmul
                        sw = min(512, S - s0)
                        ssl = slice(s0, s0 + sw)
                        nc.tensor.matmul(
                            out=lgT_ps[:, ssl], lhsT=wh_t[:, wsl],
                            rhs=xh_t[:, ssl], start=(dc == 0), stop=False)
                        nc.tensor.matmul(
                            out=lgT_ps[:, ssl], lhsT=wh_t[:, wsl],
                            rhs=xl_t[:, ssl], start=False, stop=False)
                        nc.tensor.matmul(
                            out=lgT_ps[:, ssl], lhsT=wl_t[:, wsl],
                            rhs=xh_t[:, ssl], start=False,
                            stop=(dc == DC - 1))
                lgT_sb = sb.tile([E, S], f32, tag="lgTsb", bufs=2)
                nc.scalar.copy(out=lgT_sb[:], in_=lgT_ps[:])

                # persistent per-group tiles
                lg_all = sb.tile([P, SC * E], f32, tag="lg", bufs=2)
                oh_all = sb.tile([P, 2 * SC * C], f32, tag="oh", bufs=2)
                ef_all = sb.tile([P, 2 * SC], f32, tag="ef", bufs=2)
                val_all = sb.tile([P, 2 * SC], f32, tag="val", bufs=2)
                thr_all = sb.tile([P, SC], f32, tag="thr", bufs=2)
                st_ps = ps.tile([E, 2], f32, tag="stats", bufs=1)

                # transpose logits back to [token, expert] chunks (PE)
                for j in range(SC):
                    lgt_ps = ps.tile([P, E], f32, tag="lgt", bufs=2)
                    nc.tensor.transpose(
                        out=lgt_ps[:],
                        in_=lgT_sb[:, j * P:(j + 1) * P],
                        identity=eye_t[:])
                    nc.scalar.copy(
                        out=lg_all[:, j * E:(j + 1) * E], in_=lgt_ps[:])

                # ============ per-chunk epilogue (exp phase) ============
                for j in range(SC):
                    lg = lg_all[:, j * E:(j + 1) * E]
                    tok = slice(j * P, (j + 1) * P)

                    nz = sb.tile([P, E], f32, tag="nz", bufs=3)
                    nc.sync.dma_start(out=nz[:], in_=noise_d[g, tok, :])
                    noisy = sb.tile([P, E], f32, tag="noisy", bufs=3)
                    nc.vector.scalar_tensor_tensor(
                        out=noisy[:], in0=nz[:], scalar=1.0 / SIGMA_INV,
                        in1=lg, op0=OP.mult, op1=OP.add)

                    m8 = sb.tile([P, 8], f32, tag="m8", bufs=3)
                    nc.vector.max(out=m8[:], in_=noisy[:])
                    i8 = sb.tile([P, 8], u32, tag="i8", bufs=3)
                    nc.vector.max_index(out=i8[:], in_max=m8[:], in_values=noisy[:])

                    negm1 = sb.tile([P, 1], f32, tag="negm1", bufs=3)
                    nc.vector.tensor_scalar_mul(negm1[:], m8[:, 0:1], -1.0)

                    exn = sb.tile([P, E], f32, tag="exn", bufs=3)
                    zden = sb.tile([P, 1], f32, tag="zden", bufs=3)
                    nc.scalar.activation(
                        out=exn[:], in_=noisy[:], func=AF.Exp,
                        bias=negm1[:], scale=1.0, accum_out=zden[:])
                    # v0 = r = 1/Z  (gate of the top-1 expert)
                    nc.vector.reciprocal(val_all[:, j:j + 1], zden[:])

                    gn = sb.tile([P, E], f32, tag="gn", bufs=3)
                    nc.vector.tensor_scalar_mul(gn[:], exn[:], val_all[:, j:j + 1])
                    nc.sync.dma_start(out=gates_d[g, tok, :], in_=gn[:])

                    # v1 = exp(m2 - m1) * r  (gate of the top-2 expert)
                    v1e = sb.tile([P, 1], f32, tag="v1e", bufs=3)
                    nc.scalar.activation(
                        out=v1e[:], in_=m8[:, 1:2], func=AF.Exp,
                        bias=negm1[:], scale=1.0)
                    nc.vector.tensor_tensor(
                        out=val_all[:, SC + j:SC + j + 1], in0=v1e[:],
                        in1=val_all[:, j:j + 1], op=OP.mult)

                    # clean softmax -> importance partial sums
                    negcm = sb.tile([P, 1], f32, tag="negcm", bufs=3)
                    nc.vector.tensor_reduce(
                        out=negcm[:], in_=lg, axis=AX.X, op=OP.max, negate=True)
                    cex = sb.tile([P, E], f32, tag="cex", bufs=3)
                    czden = sb.tile([P, 1], f32, tag="czden", bufs=3)
                    nc.scalar.activation(
                        out=cex[:], in_=lg, func=AF.Exp,
                        bias=negcm[:], scale=1.0, accum_out=czden[:])
                    cr = sb.tile([P, 1], f32, tag="cr", bufs=3)
                    nc.vector.reciprocal(cr[:], czden[:])
                    nc.tensor.matmul(
                        out=st_ps[:, 0:1], lhsT=cex[:], rhs=cr[:],
                        start=(j == 0), stop=False)

                    # threshold bias for the erf phase below
                    nc.vector.tensor_scalar_mul(
                        thr_all[:, j:j + 1], m8[:, 1:2], -ERF_SCALE)

                    # expert one-hots + f32 expert ids for the dispatcher
                    nc.vector.tensor_copy(out=ef_all[:, j:j + 1], in_=i8[:, 0:1])
                    nc.vector.tensor_copy(
                        out=ef_all[:, SC + j:SC + j + 1], in_=i8[:, 1:2])
                    nc.vector.tensor_scalar(
                        out=oh_all[:, j * C:(j + 1) * C], in0=iotar_t[:],
                        scalar1=ef_all[:, j:j + 1], scalar2=None,
                        op0=OP.is_equal)
                    nc.vector.tensor_scalar(
                        out=oh_all[:, (SC + j) * C:(SC + j + 1) * C],
                        in0=iotar_t[:],
                        scalar1=ef_all[:, SC + j:SC + j + 1], scalar2=None,
                        op0=OP.is_equal)

                # ============ erf phase (one ACT table switch per group) ====
                # load-loss: sum_s erf((l - thr) * 64 / sqrt(2))
                for j in range(SC):
                    erf_t = sb.tile([P, E], f32, tag="erf", bufs=3)
                    nc.scalar.activation(
                        out=erf_t[:], in_=lg_all[:, j * E:(j + 1) * E],
                        func=erf_func,
                        bias=thr_all[:, j:j + 1], scale=ERF_SCALE)
                    nc.tensor.matmul(
                        out=st_ps[:, 1:2], lhsT=erf_t[:], rhs=onesc_t[:],
                        start=False, stop=(j == SC - 1))

                # ============ dispatcher: positions + scatter ============
                # Running per-expert selection count lives in SBUF (cnt);
                # entering chunk t it holds the exclusive prefix.
                cnt = None
                for t in range(2 * SC):
                    j = t % SC
                    oh_t = oh_all[:, t * C:(t + 1) * C]
                    col_ps = ps.tile([1, E], f32, tag="col", bufs=1)
                    nc.tensor.matmul(
                        out=col_ps[:], lhsT=onesc_t[:], rhs=oh_t,
                        start=True, stop=True)
                    cs_ps = ps.tile([P, E], f32, tag="cs", bufs=2)
                    nc.tensor.matmul(
                        out=cs_ps[:], lhsT=u_t[:], rhs=oh_t,
                        start=True, stop=(t == 0))
                    if t > 0:
                        nc.tensor.matmul(
                            out=cs_ps[:], lhsT=onesr_t[:], rhs=cnt[:],
                            start=False, stop=True)
                    ncnt = sb.tile([1, E], f32, tag="cnt", bufs=3)
                    if t == 0:
                        nc.vector.tensor_copy(out=ncnt[:], in_=col_ps[:])
                    else:
                        nc.vector.tensor_tensor(
                            out=ncnt[:], in0=cnt[:], in1=col_ps[:], op=OP.add)
                    cnt = ncnt

                    tmp = sb.tile([P, E], f32, tag="postmp", bufs=3)
                    nc.vector.tensor_tensor(
                        out=tmp[:], in0=cs_ps[:], in1=oh_t, op=OP.mult)
                    pos1 = sb.tile([P, 1], f32, tag="pos1", bufs=3)
                    nc.vector.reduce_sum(out=pos1[:], in_=tmp[:], axis=AX.X)

                    # row index: e + 64*s + 65536*g ; dropped -> huge
                    idxf = sb.tile([P, 1], f32, tag="idxf", bufs=3)
                    nc.vector.scalar_tensor_tensor(
                        out=idxf[:], in0=ef_all[:, t:t + 1],
                        scalar=float(g * S * E + j * P * E),
                        in1=iotap_t[:], op0=OP.add, op1=OP.add)
                    dm = sb.tile([P, 1], f32, tag="dm", bufs=3)
                    nc.vector.tensor_scalar(
                        out=dm[:], in0=pos1[:], scalar1=float(C + 1),
                        scalar2=None, op0=OP.is_ge)
                    idxf2 = sb.tile([P, 1], f32, tag="idxf2", bufs=3)
                    # dropped rows get index 2^24 + real: > bounds_check but
                    # (idx*C) still far below 2^31 (no i32 overflow in DGE)
                    nc.vector.scalar_tensor_tensor(
                        out=idxf2[:], in0=dm[:], scalar=16777216.0, in1=idxf[:],
                        op0=OP.mult, op1=OP.add)
                    idx_i = sb.tile([P, 1], i32, tag="idxi", bufs=3)
                    nc.vector.tensor_copy(out=idx_i[:], in_=idxf2[:])

                    posc = sb.tile([P, 1], f32, tag="posc", bufs=3)
                    nc.vector.tensor_scalar(
                        out=posc[:], in0=pos1[:], scalar1=1.0, scalar2=None,
                        op0=OP.subtract)
                    rows = sb.tile([P, C], f32, tag="rows", bufs=3)
                    nc.vector.tensor_scalar(
                        out=rows[:], in0=iotar_t[:], scalar1=posc[:],
                        scalar2=val_all[:, t:t + 1], op0=OP.is_equal,
                        op1=OP.mult)

                    nc.gpsimd.indirect_dma_start(
                        out=combine_d[:],
                        out_offset=bass.IndirectOffsetOnAxis(
                            ap=idx_i[:, 0:1], axis=0),
                        in_=rows[:],
                        in_offset=None,
                        bounds_check=NROW - 1,
                        oob_is_err=False,
                    )

                # ================= losses =================
                st_sb = sb.tile([E, 2], f32, tag="st_sb", bufs=2)
                nc.vector.tensor_copy(out=st_sb[:], in_=st_ps[:])
                tr_ps = ps.tile([2, E], f32, tag="lgt", bufs=2)
                nc.tensor.matmul(
                    out=tr_ps[:], lhsT=st_sb[:], rhs=eye_t[:],
                    start=True, stop=True)
                x2 = sb.tile([2, E], f32, tag="x2", bufs=2)
                nc.vector.tensor_scalar(
                    out=x2[:], in0=tr_ps[:], scalar1=pscale_t[:],
                    scalar2=pbias_t[:], op0=OP.mult, op1=OP.add)
                mean2 = sb.tile([2, 1], f32, tag="mean2", bufs=2)
                nc.vector.reduce_sum(out=mean2[:], in_=x2[:], axis=AX.X)
                nc.vector.tensor_scalar_mul(mean2[:], mean2[:], 1.0 / E)
                dev = sb.tile([2, E], f32, tag="dev", bufs=2)
                nc.vector.tensor_scalar(
                    out=dev[:], in0=x2[:], scalar1=mean2[:], scalar2=None,
                    op0=OP.subtract)
                var2 = sb.tile([2, 1], f32, tag="var2", bufs=2)
                nc.vector.scalar_tensor_tensor(
                    out=dev[:], in0=dev[:], scalar=1.0, in1=dev[:],
                    op0=OP.mult, op1=OP.mult, accum_out=var2[:])
                nc.vector.tensor_scalar_mul(var2[:], var2[:], 1.0 / E)
                imean = sb.tile([2, 1], f32, tag="imean", bufs=2)
                nc.vector.reciprocal(imean[:], mean2[:])
                loss2 = sb.tile([2, 1], f32, tag="loss2", bufs=2)
                nc.vector.tensor_tensor(
                    out=loss2[:], in0=imean[:], in1=imean[:], op=OP.mult)
                nc.vector.tensor_tensor(
                    out=loss2[:], in0=loss2[:], in1=var2[:], op=OP.mult)
                # transpose [2,1] -> [1,2], then aux = sum
                ltr_ps = ps.tile([1, 2], f32, tag="lgt", bufs=2)
                nc.tensor.matmul(
                    out=ltr_ps[:], lhsT=loss2[:], rhs=eye_t[0:2, 0:2],
                    start=True, stop=True)
                lrow = sb.tile([1, 2], f32, tag="lrow", bufs=2)
                nc.vector.tensor_copy(out=lrow[:], in_=ltr_ps[:])
                auxv = sb.tile([1, 1], f32, tag="auxv", bufs=2)
                nc.vector.reduce_sum(out=auxv[:], in_=lrow[:], axis=AX.X)
                nc.sync.dma_start(out=imp_d[g:g + 1, :], in_=lrow[:, 0:1])
                nc.sync.dma_start(out=load_d[g:g + 1, :], in_=lrow[:, 1:2])
                nc.sync.dma_start(out=aux_d[g:g + 1, :], in_=auxv[:])

    nc.compile()
    return nc


_NC_CACHE = {}


def _get_nc(gpc, S, D, zero_fill=False):
    key = (gpc, S, D, zero_fill)
    if key not in _NC_CACHE:
        _NC_CACHE[key] = build_nc(gpc, S, D, zero_fill=zero_fill)
    return _NC_CACHE[key]


def make_in_maps(inputs, W, noise):
    import ml_dtypes
    bf = ml_dtypes.bfloat16
    G, S, D = inputs.shape
    gpc = G // N_CORES
    consts = make_consts(S, D)
    xs = np.asarray(inputs, dtype=np.float32).reshape(N_CORES, gpc, S, D)
    # host-side layout prep: [gpc, D, S] per core so the GEMM needs no
    # on-device transpose; split into bf16 (hi, lo) halves for the PE
    xT = np.ascontiguousarray(xs.transpose(0, 1, 3, 2))
    xh = xT.astype(bf)
    xl = (xT - xh.astype(np.float32)).astype(bf)
    ns = np.ascontiguousarray(
        np.asarray(noise, dtype=np.float32).reshape(N_CORES, gpc, S, E))
    Wc = np.asarray(W, dtype=np.float32)
    Wh = Wc.astype(bf)
    Wl = (Wc - Wh.astype(np.float32)).astype(bf)
    in_maps = []
    for c in range(N_CORES):
        im = {"xh": xh[c], "xl": xl[c], "noiseg": ns[c], "Wh": Wh, "Wl": Wl}
        im.update(consts)
        in_maps.append(im)
    return in_maps


def gather_outputs(results, G, S):
    gpc = G // N_CORES
    combine = np.concatenate(
        [r["combine"].reshape(gpc, S, E, C) for r in results], axis=0)
    gates = np.concatenate(
        [r["gates_noisy"].reshape(gpc, S, E) for r in results], axis=0)
    imp = np.concatenate([r["imp_out"].reshape(gpc) for r in results])
    load = np.concatenate([r["load_out"].reshape(gpc) for r in results])
    aux = np.concatenate([r["aux_out"].reshape(gpc) for r in results])
    return combine, aux, imp, load, gates


def run(inputs, W, noise, trace=False, zero_fill=False):
    m = _imports()
    G, S, D = inputs.shape
    gpc = G // N_CORES
    nc = _get_nc(gpc, S, D, zero_fill=zero_fill)
    in_maps = make_in_maps(inputs, W, noise)
    res = m["run_bass_kernel_spmd"](
        nc, in_maps, list(range(N_CORES)), trace=trace)
    outs = gather_outputs(res.results, G, S)
    return outs, res


def kernel(inputs, W, noise):
    outs, _ = run(np.asarray(inputs), np.asarray(W), np.asarray(noise))
    return outs
